# revision 2
# baseline (speedup 1.0000x reference)
"""Trainium2 Bass kernel for nn_MDU_77592879170023.

Computes, for inputs v,q [512,36,1024], att_map [512,36,36], ru [512,36]:
    mm  = LN1(v + q)
    rg  = att_map.sum(-1);  base = 0.98*ru + rg
    ru_new = base / 1.98 (or base if ru is all zero)
    rm  = focal gate;  w = rg * (8 if rm else 1/8);  ru_out similarly
    h   = relu((w*mm) @ W1 + c1);  fc = h @ W2 + c2
    out = LN2(w*mm + fc)
Returns (out [512,36,1024], ru_out [512,36]).

Strategy: pure data-parallel over the batch dim across 8 NeuronCores.
Per core: 64 batches = 2304 tokens. The FFN runs in "transposed" layout
(hidden on partitions, tokens on free) so both weight matrices are the
matmul stationary operands in their natural layouts; activations are
transposed on the PE (128x128 tiles) on the way in and out. Matmuls in
bf16 (weights pre-cast on host), everything else fp32.
"""
import numpy as np
import ml_dtypes

import concourse.bass as bass
import concourse.bacc as bacc
import concourse.tile as tile
from concourse import mybir
from concourse.bass_utils import run_bass_kernel_spmd
from concourse.masks import make_identity

BS, SRC, H = 512, 36, 1024
MID = 4 * H
NCORES = 8
B_LOC = BS // NCORES           # 64 batches per core
TOK = B_LOC * SRC              # 2304 tokens per core
P = 128
NT = TOK // P                  # 18 token tiles per core
CH_T = 2                       # token tiles per chunk
NCH = NT // CH_T               # 9 chunks
CHTOK = CH_T * P               # 256 tokens per chunk
KT = H // P                    # 8 hidden tiles
MT = MID // P                  # 32 mid tiles
ALPHA = 8.0
GAMMA = 0.98
EPS = 1e-6
F32 = mybir.dt.float32
BF16 = mybir.dt.bfloat16
OP = mybir.AluOpType
AF = mybir.ActivationFunctionType


def build_nc(ru_all_zero: bool, affine1: bool, affine2: bool):
    nc = bacc.Bacc("TRN2", target_bir_lowering=False, debug=False)

    v = nc.dram_tensor("v", [TOK, H], F32, kind="ExternalInput")
    q = nc.dram_tensor("q", [TOK, H], F32, kind="ExternalInput")
    att = nc.dram_tensor("att", [B_LOC, SRC, SRC], F32, kind="ExternalInput")
    ru = nc.dram_tensor("ru", [B_LOC, SRC], F32, kind="ExternalInput")
    w1 = nc.dram_tensor("w1", [KT, MT, P, P], BF16, kind="ExternalInput")
    w2 = nc.dram_tensor("w2", [MT, KT, P, P], BF16, kind="ExternalInput")
    c1r = nc.dram_tensor("c1r", [P, MT], F32, kind="ExternalInput")
    c2r = nc.dram_tensor("c2r", [P, KT], F32, kind="ExternalInput")
    if affine1:
        g1 = nc.dram_tensor("g1", [H], F32, kind="ExternalInput")
        b1 = nc.dram_tensor("b1", [H], F32, kind="ExternalInput")
    if affine2:
        g2 = nc.dram_tensor("g2", [H], F32, kind="ExternalInput")
        b2 = nc.dram_tensor("b2", [H], F32, kind="ExternalInput")
    out = nc.dram_tensor("out", [TOK, H], F32, kind="ExternalOutput")
    ru_out = nc.dram_tensor("ru_out", [B_LOC, SRC], F32, kind="ExternalOutput")

    with tile.TileContext(nc) as tc:
        with (
            tc.tile_pool(name="singles", bufs=1) as singles,
            tc.tile_pool(name="wpool", bufs=1) as wpool,
            tc.tile_pool(name="gate", bufs=1) as gate,
            tc.tile_pool(name="upool", bufs=4) as upool,
            tc.tile_pool(name="qpool", bufs=2) as qpool,
            tc.tile_pool(name="opool", bufs=2) as opool,
            tc.tile_pool(name="tpool", bufs=1) as tpool,
            tc.tile_pool(name="hpool", bufs=1) as hpool,
            tc.tile_pool(name="fpool", bufs=1) as fpool,
            tc.tile_pool(name="stats", bufs=8) as stats,
            tc.tile_pool(name="mmps", bufs=4, space="PSUM") as mmps,
            tc.tile_pool(name="tps", bufs=4, space="PSUM") as tps,
            tc.tile_pool(name="dram", bufs=1, space="DRAM") as drampool,
        ):
            # ---- resident weights ----
            w1_sb = wpool.tile([P, KT, MT, P], BF16)
            for m in range(MT):
                nc.sync.dma_start(
                    out=w1_sb[:, :, m, :],
                    in_=w1[:, m, :, :].rearrange("k p f -> p k f"),
                )
            w2_sb = wpool.tile([P, MT, KT, P], BF16)
            for k in range(KT):
                for half in range(2):
                    ms = slice(half * MT // 2, (half + 1) * MT // 2)
                    nc.sync.dma_start(
                        out=w2_sb[:, ms, k, :],
                        in_=w2[ms, k, :, :].rearrange("m p f -> p m f"),
                    )
            c1_sb = singles.tile([P, MT], F32)
            nc.sync.dma_start(out=c1_sb, in_=c1r[:, :])
            c2_sb = singles.tile([P, KT], F32)
            nc.sync.dma_start(out=c2_sb, in_=c2r[:, :])

            eps_sb = singles.tile([P, 1], F32)
            nc.vector.memset(eps_sb, EPS)

            ident0 = singles.tile([P, P], F32)
            make_identity(nc, ident0)
            ident = singles.tile([P, P], F32)
            nc.vector.tensor_copy(out=ident, in_=ident0)

            def bcast_row(dram_t):
                t = singles.tile([P, H], F32)
                src = bass.AP(tensor=dram_t.ap().tensor, offset=0,
                              ap=[[0, P], [1, H]])
                nc.gpsimd.dma_start(out=t, in_=src)
                return t

            g1b = bcast_row(g1) if affine1 else None
            b1b = bcast_row(b1) if affine1 else None
            g2b = bcast_row(g2) if affine2 else None
            b2b = bcast_row(b2) if affine2 else None

            # ---- gate phase: rg, ru_new, focal gate, w, ru_out ----
            att_sb = gate.tile([B_LOC, SRC, SRC], F32)
            nc.sync.dma_start(out=att_sb, in_=att[:, :, :])
            ru_sb = gate.tile([B_LOC, SRC], F32)
            nc.sync.dma_start(out=ru_sb, in_=ru[:, :])

            rg = gate.tile([B_LOC, SRC], F32)
            nc.vector.tensor_reduce(out=rg, in_=att_sb, axis=mybir.AxisListType.X, op=OP.add)
            base = gate.tile([B_LOC, SRC], F32)
            nc.vector.scalar_tensor_tensor(out=base, in0=ru_sb, scalar=GAMMA, in1=rg,
                                           op0=OP.mult, op1=OP.add)
            ru_new = gate.tile([B_LOC, SRC], F32)
            if ru_all_zero:
                nc.vector.tensor_copy(out=ru_new, in_=base)
            else:
                # base / (1+GAMMA) as double-float multiply to match fp32 divide
                r = 1.0 / (np.float64(np.float32(1.0) + np.float32(GAMMA)))
                r0 = np.float32(r)
                r1 = np.float32(r - np.float64(r0))
                t1 = gate.tile([B_LOC, SRC], F32)
                nc.vector.tensor_single_scalar(out=t1, in_=base, scalar=float(r1), op=OP.mult)
                nc.vector.scalar_tensor_tensor(out=ru_new, in0=base, scalar=float(r0),
                                               in1=t1, op0=OP.mult, op1=OP.add)
            sa = gate.tile([B_LOC, SRC], F32)
            nc.scalar.activation(out=sa, in_=ru_new, func=AF.Sqrt)
            asa = gate.tile([B_LOC, SRC], F32)
            nc.vector.tensor_mul(out=asa, in0=ru_new, in1=sa)
            S = gate.tile([B_LOC, 1], F32)
            nc.vector.tensor_reduce(out=S, in_=sa, axis=mybir.AxisListType.X, op=OP.add)
            T2 = gate.tile([B_LOC, 1], F32)
            nc.vector.tensor_reduce(out=T2, in_=asa, axis=mybir.AxisListType.X, op=OP.add)
            funcF = gate.tile([B_LOC, SRC], F32)
            nc.vector.tensor_scalar(out=funcF, in0=ru_new, scalar1=S, scalar2=T2,
                                    op0=OP.mult, op1=OP.subtract)
            rm = gate.tile([B_LOC, SRC], F32)
            nc.vector.tensor_single_scalar(out=rm, in_=funcF, scalar=0.0, op=OP.is_gt)
            factor = gate.tile([B_LOC, SRC], F32)
            nc.vector.tensor_scalar(out=factor, in0=rm, scalar1=ALPHA - 1.0 / ALPHA,
                                    scalar2=1.0 / ALPHA, op0=OP.mult, op1=OP.add)
            ru_out_t = gate.tile([B_LOC, SRC], F32)
            nc.vector.tensor_mul(out=ru_out_t, in0=ru_new, in1=factor)
            nc.sync.dma_start(out=ru_out[:, :], in_=ru_out_t)
            w_t = gate.tile([B_LOC, SRC], F32)
            nc.vector.tensor_mul(out=w_t, in0=rg, in1=factor)
            w_dram = drampool.tile([TOK], F32)
            nc.sync.dma_start(out=w_dram.rearrange("(a b) -> a b", b=SRC), in_=w_t)
            w_sb = singles.tile([P, NT], F32)
            nc.sync.dma_start(out=w_sb, in_=w_dram.rearrange("(n p) -> p n", p=P))

            # ---- main pipeline over chunks ----
            for c in range(NCH):
                u_tiles = []
                mmT = [tpool.tile([P, CHTOK], BF16, tag=f"mmT{k}", name=f"mmT{k}_c{c}") for k in range(KT)]
                for j in range(CH_T):
                    i = c * CH_T + j
                    rows = slice(i * P, (i + 1) * P)
                    u = upool.tile([P, H], F32, tag="u")
                    nc.sync.dma_start(out=u, in_=v[rows, :])
                    qt = qpool.tile([P, H], F32, tag="q")
                    nc.sync.dma_start(out=qt, in_=q[rows, :])
                    nc.vector.tensor_add(out=u, in0=u, in1=qt)

                    st = stats.tile([P, 2, 6], F32, tag="st")
                    nc.vector.bn_stats(out=st[:, 0, :], in_=u[:, 0:512])
                    nc.vector.bn_stats(out=st[:, 1, :], in_=u[:, 512:1024])
                    mv = stats.tile([P, 2], F32, tag="mv")
                    nc.vector.bn_aggr(out=mv, in_=st)
                    rstd = stats.tile([P, 1], F32, tag="rstd")
                    nc.scalar.activation(out=rstd, in_=mv[:, 1:2], func=AF.Sqrt,
                                         bias=eps_sb, scale=1.0)
                    nc.vector.reciprocal(out=rstd, in_=rstd)
                    rstd_w = stats.tile([P, 1], F32, tag="rstd_w")
                    nc.vector.tensor_mul(out=rstd_w, in0=rstd, in1=w_sb[:, i:i + 1])
                    if affine1:
                        # u <- (u - mu) * rstd (LN core), then *g1 *w + w*b1
                        nc.vector.tensor_scalar(out=u, in0=u, scalar1=mv[:, 0:1],
                                                scalar2=rstd, op0=OP.subtract, op1=OP.mult)
                        nc.vector.tensor_mul(out=u, in0=u, in1=g1b)
                        nc.vector.tensor_scalar_add(out=u, in0=u, scalar1=...)  # placeholder
                        raise NotImplementedError("general b1 path below")
                    else:
                        nc.vector.tensor_scalar(out=u, in0=u, scalar1=mv[:, 0:1],
                                                scalar2=rstd_w, op0=OP.subtract, op1=OP.mult)
                    u_tiles.append(u)

                    for k in range(KT):
                        pst = tps.tile([P, P], F32, tag="tp")
                        nc.tensor.transpose(pst, u[:, k * P:(k + 1) * P], ident)
                        nc.vector.tensor_copy(out=mmT[k][:, j * P:(j + 1) * P], in_=pst)

                # FFN layer 1: hT[m] = relu(W1[:,m].T @ mmT + c1[m])
                hT = [hpool.tile([P, CHTOK], BF16, tag=f"hT{m}", name=f"hT{m}_c{c}") for m in range(MT)]
                for m in range(MT):
                    ps = mmps.tile([P, CHTOK], F32, tag="mmps")
                    for k in range(KT):
                        nc.tensor.matmul(ps, w1_sb[:, k, m, :], mmT[k],
                                         start=(k == 0), stop=(k == KT - 1))
                    nc.scalar.activation(out=hT[m], in_=ps, func=AF.Relu,
                                         bias=c1_sb[:, m:m + 1], scale=1.0)

                # FFN layer 2: fcT[k] = W2[:,k].T @ hT + c2[k]
                fcT = [fpool.tile([P, CHTOK], F32, tag=f"fcT{k}", name=f"fcT{k}_c{c}") for k in range(KT)]
                for k in range(KT):
                    ps2 = mmps.tile([P, CHTOK], F32, tag="mmps")
                    for m in range(MT):
                        nc.tensor.matmul(ps2, w2_sb[:, m, k, :], hT[m],
                                         start=(m == 0), stop=(m == MT - 1))
                    nc.vector.tensor_scalar(out=fcT[k], in0=ps2, scalar1=c2_sb[:, k:k + 1],
                                            scalar2=None, op0=OP.add)

                # back-transpose, residual, LN2, store
                for j in range(CH_T):
                    i = c * CH_T + j
                    rows = slice(i * P, (i + 1) * P)
                    u = u_tiles[j]
                    for k in range(KT):
                        psb = tps.tile([P, P], F32, tag="tp")
                        nc.tensor.transpose(psb, fcT[k][:, j * P:(j + 1) * P], ident)
                        nc.vector.tensor_add(out=u[:, k * P:(k + 1) * P], in0=psb,
                                             in1=u[:, k * P:(k + 1) * P])
                    st2 = stats.tile([P, 2, 6], F32, tag="st")
                    nc.vector.bn_stats(out=st2[:, 0, :], in_=u[:, 0:512])
                    nc.vector.bn_stats(out=st2[:, 1, :], in_=u[:, 512:1024])
                    mv2 = stats.tile([P, 2], F32, tag="mv")
                    nc.vector.bn_aggr(out=mv2, in_=st2)
                    rstd2 = stats.tile([P, 1], F32, tag="rstd")
                    nc.scalar.activation(out=rstd2, in_=mv2[:, 1:2], func=AF.Sqrt,
                                         bias=eps_sb, scale=1.0)
                    nc.vector.reciprocal(out=rstd2, in_=rstd2)
                    ot = opool.tile([P, H], F32, tag="o")
                    nc.vector.tensor_scalar(out=ot, in0=u, scalar1=mv2[:, 0:1],
                                            scalar2=rstd2, op0=OP.subtract, op1=OP.mult)
                    if affine2:
                        nc.vector.tensor_mul(out=ot, in0=ot, in1=g2b)
                        nc.vector.tensor_add(out=ot, in0=ot, in1=b2b)
                    nc.sync.dma_start(out=out[rows, :], in_=ot)

    nc.compile()
    return nc


_cache: dict = {}


def _get_nc(key):
    if key not in _cache:
        _cache[key] = build_nc(*key)
    return _cache[key]


def kernel(**inputs) -> tuple:
    v = np.asarray(inputs["v"], dtype=np.float32)
    q = np.asarray(inputs["q"], dtype=np.float32)
    att_map = np.asarray(inputs["att_map"], dtype=np.float32)
    ru = np.asarray(inputs["ru"], dtype=np.float32)
    g1 = np.asarray(inputs["g1"], dtype=np.float32)
    b1 = np.asarray(inputs["b1"], dtype=np.float32)
    g2 = np.asarray(inputs["g2"], dtype=np.float32)
    b2 = np.asarray(inputs["b2"], dtype=np.float32)
    W1 = np.asarray(inputs["W1"], dtype=np.float32)
    c1 = np.asarray(inputs["c1"], dtype=np.float32)
    W2 = np.asarray(inputs["W2"], dtype=np.float32)
    c2 = np.asarray(inputs["c2"], dtype=np.float32)

    ru_all_zero = not np.any(ru)
    affine1 = not (np.all(g1 == 1.0) and np.all(b1 == 0.0))
    affine2 = not (np.all(g2 == 1.0) and np.all(b2 == 0.0))
    if affine1:
        raise NotImplementedError("nontrivial LN1 affine params not supported")

    nc = _get_nc((ru_all_zero, affine1, affine2))

    # host-side weight prep (replicated across cores)
    w1t = np.ascontiguousarray(
        W1.reshape(KT, P, MT, P).transpose(0, 2, 1, 3)).astype(ml_dtypes.bfloat16)
    w2t = np.ascontiguousarray(
        W2.reshape(MT, P, KT, P).transpose(0, 2, 1, 3)).astype(ml_dtypes.bfloat16)
    c1r = np.ascontiguousarray(c1.reshape(MT, P).T)
    c2r = np.ascontiguousarray(c2.reshape(KT, P).T)

    v8 = v.reshape(NCORES, TOK, H)
    q8 = q.reshape(NCORES, TOK, H)
    att8 = att_map.reshape(NCORES, B_LOC, SRC, SRC)
    ru8 = ru.reshape(NCORES, B_LOC, SRC)

    in_maps = []
    for c in range(NCORES):
        m = {
            "v": np.ascontiguousarray(v8[c]),
            "q": np.ascontiguousarray(q8[c]),
            "att": np.ascontiguousarray(att8[c]),
            "ru": np.ascontiguousarray(ru8[c]),
            "w1": w1t, "w2": w2t, "c1r": c1r, "c2r": c2r,
        }
        if affine2:
            m["g2"] = g2
            m["b2"] = b2
        in_maps.append(m)

    res = run_bass_kernel_spmd(nc, in_maps, core_ids=list(range(NCORES)))
    out = np.concatenate(
        [r["out"].reshape(B_LOC, SRC, H) for r in res.results], axis=0)
    ru_out = np.concatenate([r["ru_out"] for r in res.results], axis=0)
    return out, ru_out


# revision 10
# speedup vs baseline: 1.9399x; 1.9399x over previous
"""Trainium2 Bass kernel for nn_MDU_77592879170023.

Computes, for inputs v,q [512,36,1024], att_map [512,36,36], ru [512,36]:
    mm  = LN1(v + q)
    rg  = att_map.sum(-1);  base = 0.98*ru + rg
    ru_new = base / 1.98 (or base if ru is all zero)
    rm  = focal gate;  w = rg * (8 if rm else 1/8);  ru_out similarly
    h   = relu((w*mm) @ W1 + c1);  fc = h @ W2 + c2
    out = LN2(w*mm + fc)
Returns (out [512,36,1024], ru_out [512,36]).

Strategy: pure data-parallel over the batch dim across 8 NeuronCores.
Per core: 64 batches = 2304 tokens. The FFN runs in "transposed" layout
(hidden on partitions, tokens on free) so both weight matrices are the
matmul stationary operands in their natural layouts; activations are
transposed on the PE (128x128 tiles) on the way in and out. Matmuls in
bf16 (weights pre-cast on host), everything else fp32. The chunk loop
is software-pipelined: the next chunk's LayerNorm + forward transposes
are issued between this chunk's two matmul layers so the vector engine
never head-of-line-blocks the PE at chunk boundaries.
"""
import numpy as np
import ml_dtypes

import concourse.bass as bass
import concourse.bacc as bacc
import concourse.tile as tile
from concourse import mybir
from concourse.bass_utils import run_bass_kernel_spmd
from concourse.masks import make_identity

BS, SRC, H = 512, 36, 1024
MID = 4 * H
NCORES = 8
B_LOC = BS // NCORES           # 64 batches per core
TOK = B_LOC * SRC              # 2304 tokens per core
P = 128
NT = TOK // P                  # 18 token tiles per core
CH_T = 2                       # token tiles per chunk
NCH = NT // CH_T               # 9 chunks
CHTOK = CH_T * P               # 256 tokens per chunk
KT = H // P                    # 8 hidden tiles
MT = MID // P                  # 32 mid tiles
ALPHA = 8.0
GAMMA = 0.98
EPS = 1e-6
F32 = mybir.dt.float32
BF16 = mybir.dt.bfloat16
OP = mybir.AluOpType
AF = mybir.ActivationFunctionType


def build_nc(ru_all_zero: bool, affine2: bool):
    nc = bacc.Bacc("TRN2", target_bir_lowering=False, debug=False)

    v = nc.dram_tensor("v", [TOK, H], F32, kind="ExternalInput")
    q = nc.dram_tensor("q", [TOK, H], F32, kind="ExternalInput")
    att = nc.dram_tensor("att", [B_LOC, SRC, SRC], F32, kind="ExternalInput")
    ru = nc.dram_tensor("ru", [B_LOC, SRC], F32, kind="ExternalInput")
    w1 = nc.dram_tensor("w1", [MT, P, KT, P], BF16, kind="ExternalInput")
    w2 = nc.dram_tensor("w2", [KT, P, MT, P], BF16, kind="ExternalInput")
    c1r = nc.dram_tensor("c1r", [P, MT], F32, kind="ExternalInput")
    c2r = nc.dram_tensor("c2r", [P, KT], F32, kind="ExternalInput")
    if affine2:
        g2 = nc.dram_tensor("g2", [H], F32, kind="ExternalInput")
        b2 = nc.dram_tensor("b2", [H], F32, kind="ExternalInput")
    out = nc.dram_tensor("out", [TOK, H], F32, kind="ExternalOutput")
    ru_out = nc.dram_tensor("ru_out", [B_LOC, SRC], F32, kind="ExternalOutput")

    with tile.TileContext(nc) as tc:
        with (
            tc.tile_pool(name="singles", bufs=1) as singles,
            tc.tile_pool(name="wpool", bufs=1) as wpool,
            tc.tile_pool(name="gate", bufs=1) as gate,
            tc.tile_pool(name="upool", bufs=5) as upool,
            tc.tile_pool(name="qpool", bufs=2) as qpool,
            tc.tile_pool(name="opool", bufs=2) as opool,
            tc.tile_pool(name="tpool", bufs=2) as tpool,
            tc.tile_pool(name="hpool", bufs=1) as hpool,
            tc.tile_pool(name="fpool", bufs=1) as fpool,
            tc.tile_pool(name="stats", bufs=8) as stats,
            tc.tile_pool(name="mmps", bufs=4, space="PSUM") as mmps,
            tc.tile_pool(name="tps", bufs=4, space="PSUM") as tps,
            tc.tile_pool(name="dram", bufs=1, space="DRAM") as drampool,
        ):
            # ---- resident weight tiles (DMAs issued later, after the gate
            # phase and first chunks' input DMAs, so the critical path isn't
            # queued behind 16.7MB of weights) ----
            w1m = [wpool.tile([P, KT, P], BF16, name=f"w1m{m}") for m in range(MT)]
            w2k = [wpool.tile([P, MT, P], BF16, name=f"w2k{k}") for k in range(KT)]

            def load_weights():
                # host layouts are partition-major: per-partition runs are
                # 2KB (w1) / 8KB (w2) contiguous, one DMA per tile
                for m in range(MT):
                    nc.sync.dma_start(out=w1m[m], in_=w1[m, :, :, :])
                for k in range(KT):
                    nc.sync.dma_start(out=w2k[k], in_=w2[k, :, :, :])

            c1_sb = singles.tile([P, MT], F32)
            nc.sync.dma_start(out=c1_sb, in_=c1r[:, :])
            c2_sb = singles.tile([P, KT], F32)
            nc.sync.dma_start(out=c2_sb, in_=c2r[:, :])

            eps_sb = singles.tile([P, 1], F32)
            nc.vector.memset(eps_sb, EPS)

            ident0 = singles.tile([P, P], F32)
            make_identity(nc, ident0)
            ident = singles.tile([P, P], F32)
            nc.vector.tensor_copy(out=ident, in_=ident0)

            if affine2:
                g2b = singles.tile([P, H], F32)
                nc.gpsimd.dma_start(out=g2b, in_=bass.AP(
                    tensor=g2.ap().tensor, offset=0, ap=[[0, P], [1, H]]))
                b2b = singles.tile([P, H], F32)
                nc.gpsimd.dma_start(out=b2b, in_=bass.AP(
                    tensor=b2.ap().tensor, offset=0, ap=[[0, P], [1, H]]))

            # ---- gate phase: rg, ru_new, focal gate, w, ru_out ----
            att_sb = gate.tile([B_LOC, SRC, SRC], F32)
            nc.sync.dma_start(out=att_sb, in_=att[:, :, :])
            ru_sb = gate.tile([B_LOC, SRC], F32)
            nc.sync.dma_start(out=ru_sb, in_=ru[:, :])

            rg = gate.tile([B_LOC, SRC], F32)
            nc.vector.tensor_reduce(out=rg, in_=att_sb, axis=mybir.AxisListType.X, op=OP.add)
            base = gate.tile([B_LOC, SRC], F32)
            nc.vector.scalar_tensor_tensor(out=base, in0=ru_sb, scalar=GAMMA, in1=rg,
                                           op0=OP.mult, op1=OP.add)
            ru_new = gate.tile([B_LOC, SRC], F32)
            if ru_all_zero:
                nc.vector.tensor_copy(out=ru_new, in_=base)
            else:
                # base / (1+GAMMA) as double-float multiply to match fp32 divide
                r = 1.0 / (np.float64(np.float32(1.0) + np.float32(GAMMA)))
                r0 = np.float32(r)
                r1 = np.float32(r - np.float64(r0))
                t1 = gate.tile([B_LOC, SRC], F32)
                nc.vector.tensor_single_scalar(out=t1, in_=base, scalar=float(r1), op=OP.mult)
                nc.vector.scalar_tensor_tensor(out=ru_new, in0=base, scalar=float(r0),
                                               in1=t1, op0=OP.mult, op1=OP.add)
            sa = gate.tile([B_LOC, SRC], F32)
            nc.scalar.activation(out=sa, in_=ru_new, func=AF.Sqrt)
            asa = gate.tile([B_LOC, SRC], F32)
            nc.vector.tensor_mul(out=asa, in0=ru_new, in1=sa)
            S = gate.tile([B_LOC, 1], F32)
            nc.vector.tensor_reduce(out=S, in_=sa, axis=mybir.AxisListType.X, op=OP.add)
            T2 = gate.tile([B_LOC, 1], F32)
            nc.vector.tensor_reduce(out=T2, in_=asa, axis=mybir.AxisListType.X, op=OP.add)
            funcF = gate.tile([B_LOC, SRC], F32)
            nc.vector.tensor_scalar(out=funcF, in0=ru_new, scalar1=S, scalar2=T2,
                                    op0=OP.mult, op1=OP.subtract)
            rm = gate.tile([B_LOC, SRC], F32)
            nc.vector.tensor_single_scalar(out=rm, in_=funcF, scalar=0.0, op=OP.is_gt)
            factor = gate.tile([B_LOC, SRC], F32)
            nc.vector.tensor_scalar(out=factor, in0=rm, scalar1=ALPHA - 1.0 / ALPHA,
                                    scalar2=1.0 / ALPHA, op0=OP.mult, op1=OP.add)
            ru_out_t = gate.tile([B_LOC, SRC], F32)
            nc.vector.tensor_mul(out=ru_out_t, in0=ru_new, in1=factor)
            nc.sync.dma_start(out=ru_out[:, :], in_=ru_out_t)
            w_t = gate.tile([B_LOC, SRC], F32)
            nc.vector.tensor_mul(out=w_t, in0=rg, in1=factor)
            w_dram = drampool.tile([TOK], F32)
            nc.sync.dma_start(out=w_dram.rearrange("(a b) -> a b", b=SRC), in_=w_t)
            w_sb = singles.tile([P, NT], F32)
            nc.sync.dma_start(out=w_sb, in_=w_dram.rearrange("(n p) -> p n", p=P))

            # ---- software-pipelined main loop ----
            st_u = {}     # chunk -> [u tiles]  (natural-layout w*LN1, later z)
            st_mmT = {}   # chunk -> [mmT tiles]

            def stage_ln1(c):
                """DMA v/q and compute u = w * LN1(v+q) for both tiles of chunk c."""
                us = []
                for j in range(CH_T):
                    i = c * CH_T + j
                    rows = slice(i * P, (i + 1) * P)
                    u = upool.tile([P, H], F32, tag="u", name=f"u_{i}")
                    nc.sync.dma_start(out=u, in_=v[rows, :])
                    qt = qpool.tile([P, H], F32, tag="q", name=f"q_{i}")
                    nc.sync.dma_start(out=qt, in_=q[rows, :])
                    nc.vector.tensor_add(out=u, in0=u, in1=qt)
                    st = stats.tile([P, 2, 6], F32, tag="st", name=f"st_{i}")
                    nc.vector.bn_stats(out=st[:, 0, :], in_=u[:, 0:512])
                    nc.vector.bn_stats(out=st[:, 1, :], in_=u[:, 512:1024])
                    mv = stats.tile([P, 2], F32, tag="mv", name=f"mv_{i}")
                    nc.vector.bn_aggr(out=mv, in_=st)
                    rstd = stats.tile([P, 1], F32, tag="rstd", name=f"rstd_{i}")
                    nc.scalar.activation(out=rstd, in_=mv[:, 1:2], func=AF.Sqrt,
                                         bias=eps_sb, scale=1.0)
                    nc.vector.reciprocal(out=rstd, in_=rstd)
                    rstd_w = stats.tile([P, 1], F32, tag="rstd_w", name=f"rstdw_{i}")
                    nc.vector.tensor_mul(out=rstd_w, in0=rstd, in1=w_sb[:, i:i + 1])
                    nc.vector.tensor_scalar(out=u, in0=u, scalar1=mv[:, 0:1],
                                            scalar2=rstd_w, op0=OP.subtract, op1=OP.mult)
                    us.append(u)
                st_u[c] = us

            def stage_transpose(c):
                """PE-transpose u into bf16 mmT tiles for chunk c."""
                mmT = [tpool.tile([P, CHTOK], BF16, tag=f"mmT{k}", name=f"mmT{k}_c{c}")
                       for k in range(KT)]
                for j in range(CH_T):
                    u = st_u[c][j]
                    for k in range(KT):
                        pst = tps.tile([P, P], F32, tag="tp", name=f"tp_{c}_{j}_{k}")
                        nc.tensor.transpose(pst, u[:, k * P:(k + 1) * P], ident)
                        nc.vector.tensor_copy(out=mmT[k][:, j * P:(j + 1) * P], in_=pst)
                st_mmT[c] = mmT

            def stage_mm1(c):
                hT = [hpool.tile([P, CHTOK], BF16, tag=f"hT{m}", name=f"hT{m}_c{c}")
                      for m in range(MT)]
                mmT = st_mmT[c]
                for m in range(MT):
                    ps = mmps.tile([P, CHTOK], F32, tag="mmps", name=f"ps1_{c}_{m}")
                    for k in range(KT):
                        nc.tensor.matmul(ps, w1m[m][:, k, :], mmT[k],
                                         start=(k == 0), stop=(k == KT - 1))
                    nc.scalar.activation(out=hT[m], in_=ps, func=AF.Relu,
                                         bias=c1_sb[:, m:m + 1], scale=1.0)
                return hT

            def stage_mm2(c, hT):
                fcT = [fpool.tile([P, CHTOK], F32, tag=f"fcT{k}", name=f"fcT{k}_c{c}")
                       for k in range(KT)]
                for k in range(KT):
                    ps2 = mmps.tile([P, CHTOK], F32, tag="mmps", name=f"ps2_{c}_{k}")
                    for m in range(MT):
                        nc.tensor.matmul(ps2, w2k[k][:, m, :], hT[m],
                                         start=(m == 0), stop=(m == MT - 1))
                    nc.scalar.activation(out=fcT[k], in_=ps2, func=AF.Identity,
                                         bias=c2_sb[:, k:k + 1], scale=1.0)
                return fcT

            def stage_residual(c, fcT):
                """Back-transpose fc and add into u (z = w*mm + fc)."""
                for j in range(CH_T):
                    u = st_u[c][j]
                    for k in range(KT):
                        psb = tps.tile([P, P], F32, tag="tp", name=f"bt_{c}_{j}_{k}")
                        nc.tensor.transpose(psb, fcT[k][:, j * P:(j + 1) * P], ident)
                        nc.vector.tensor_add(out=u[:, k * P:(k + 1) * P], in0=psb,
                                             in1=u[:, k * P:(k + 1) * P])

            def stage_ln2(c):
                for j in range(CH_T):
                    i = c * CH_T + j
                    rows = slice(i * P, (i + 1) * P)
                    u = st_u[c][j]
                    st2 = stats.tile([P, 2, 6], F32, tag="st", name=f"st2_{i}")
                    nc.vector.bn_stats(out=st2[:, 0, :], in_=u[:, 0:512])
                    nc.vector.bn_stats(out=st2[:, 1, :], in_=u[:, 512:1024])
                    mv2 = stats.tile([P, 2], F32, tag="mv", name=f"mv2_{i}")
                    nc.vector.bn_aggr(out=mv2, in_=st2)
                    rstd2 = stats.tile([P, 1], F32, tag="rstd", name=f"rstd2_{i}")
                    nc.scalar.activation(out=rstd2, in_=mv2[:, 1:2], func=AF.Sqrt,
                                         bias=eps_sb, scale=1.0)
                    nc.vector.reciprocal(out=rstd2, in_=rstd2)
                    ot = opool.tile([P, H], F32, tag="o", name=f"o_{i}")
                    nc.vector.tensor_scalar(out=ot, in0=u, scalar1=mv2[:, 0:1],
                                            scalar2=rstd2, op0=OP.subtract, op1=OP.mult)
                    if affine2:
                        nc.vector.tensor_mul(out=ot, in0=ot, in1=g2b)
                        nc.vector.tensor_add(out=ot, in0=ot, in1=b2b)
                    nc.sync.dma_start(out=out[rows, :], in_=ot)
                del st_u[c]

            # prologue: first two chunks' inputs + transposes go ahead of the
            # bulk weight DMAs
            stage_ln1(0)
            stage_ln1(1)
            stage_transpose(0)
            load_weights()
            for c in range(NCH):
                if c > 0:
                    stage_ln2(c - 1)
                if c + 1 < NCH and c >= 1:
                    stage_ln1(c + 1)
                hT = stage_mm1(c)
                if c + 1 < NCH:
                    stage_transpose(c + 1)
                del st_mmT[c]
                fcT = stage_mm2(c, hT)
                stage_residual(c, fcT)
            stage_ln2(NCH - 1)

    nc.compile()
    return nc


_cache: dict = {}


def _get_nc(key):
    if key not in _cache:
        _cache[key] = build_nc(*key)
    return _cache[key]


def _prep_in_maps(inputs):
    np_in = {k: np.asarray(v, dtype=np.float32) for k, v in inputs.items()}
    ru_all_zero = not np.any(np_in["ru"])
    affine1 = not (np.all(np_in["g1"] == 1.0) and np.all(np_in["b1"] == 0.0))
    affine2 = not (np.all(np_in["g2"] == 1.0) and np.all(np_in["b2"] == 0.0))
    if affine1:
        raise NotImplementedError("nontrivial LN1 affine params not supported")

    # [m, p(contraction within k-tile), k, f] / [k, p, m, f]: per (tile,
    # partition) the whole free row is contiguous -> efficient DMA
    w1t = np.ascontiguousarray(
        np_in["W1"].reshape(KT, P, MT, P).transpose(2, 1, 0, 3)).astype(ml_dtypes.bfloat16)
    w2t = np.ascontiguousarray(
        np_in["W2"].reshape(MT, P, KT, P).transpose(2, 1, 0, 3)).astype(ml_dtypes.bfloat16)
    c1r = np.ascontiguousarray(np_in["c1"].reshape(MT, P).T)
    c2r = np.ascontiguousarray(np_in["c2"].reshape(KT, P).T)

    v8 = np_in["v"].reshape(NCORES, TOK, H)
    q8 = np_in["q"].reshape(NCORES, TOK, H)
    att8 = np_in["att_map"].reshape(NCORES, B_LOC, SRC, SRC)
    ru8 = np_in["ru"].reshape(NCORES, B_LOC, SRC)

    in_maps = []
    for c in range(NCORES):
        m = {
            "v": np.ascontiguousarray(v8[c]),
            "q": np.ascontiguousarray(q8[c]),
            "att": np.ascontiguousarray(att8[c]),
            "ru": np.ascontiguousarray(ru8[c]),
            "w1": w1t, "w2": w2t, "c1r": c1r, "c2r": c2r,
        }
        if affine2:
            m["g2"] = np_in["g2"]
            m["b2"] = np_in["b2"]
        in_maps.append(m)
    return (ru_all_zero, affine2), in_maps


def kernel(**inputs) -> tuple:
    key, in_maps = _prep_in_maps(inputs)
    nc = _get_nc(key)
    res = run_bass_kernel_spmd(nc, in_maps, core_ids=list(range(NCORES)))
    out = np.concatenate(
        [r["out"].reshape(B_LOC, SRC, H) for r in res.results], axis=0)
    ru_out = np.concatenate([r["ru_out"] for r in res.results], axis=0)
    return out, ru_out
